# revision 34
# baseline (speedup 1.0000x reference)
"""Trainium2 Bass kernel for mixed Gaussian/Gabor splat rasterization.

Problem: render 3072 plain 2D gaussians + 1024 gabor-modulated gaussians
(G=4 cosine carriers each) densely into a [1,3,256,256] image, clamp [0,1].

v3 design (carrier-major, host-heavy, adaptive blocks):
  - Pixels sharded across 8 cores: core k owns rows [32k, 32k+32), split into
    16 column POSITIONS of 16x32 px.  A position whose densest core has more
    than 128 overlapping lanes is split into two 8x32 SEGMENTS, so (for these
    inputs) every segment's lanes fit one 128-partition chunk and no pixel is
    rasterized twice.  Every engine op's cost scales with its free size
    (pixels), not partitions, so this halves the dense positions' work.
  - Lanes: a low gaussian is 1 lane; a high gaussian is 4 carrier lanes (one
    per gabor carrier), each carrying its own copy of the parent's sigma
    rows.  Selection is contribution-based: keep a gaussian in a segment if
    amp * exp(-sigma(closest point)) >= EPS, where amp is |color|max (low) or
    |color|max * sum|wg| (high).
  - All matmul lhsT data is precomputed on the host in f64:
      sigma [12,128]: bf16 hi/lo split weights [w0h,w0l..w5h,w5l] against
        basis rows [x2,x2,xy,xy,y2,y2,x,x,y,y,1,1] (basis values are exact in
        f32r), with the constant w5 folded in so Exp needs no bias.
      phase [6,128]: [fx,fy,o0..o3] half-angle cycles/px + per-quarter
        rounded offsets (one-hot quarter rows in the pixel basis) keep the
        activation argument within +-0.38 cycles.  The Sin activation is
        renamed to the HW table function Sin2pi (= sin(2*pi*x), measured
        exact for |x| <= 0.5) in the BIR JSON at serialization time;
        Sin2pi lives in the same act-table set as Exp ("exp_and_friends"),
        so the whole kernel needs ONE table load instead of two per pass.
  - cos via half angle with the -2 folded into the colors: pass A computes
    m = sin^2(pi t) (Sin -> bf16, square on DVE in 2x mode); pass B merges
    w = (m - 0.5) * env, and a carrier unit's single color matmul uses
    c = -2*col*wg*255 (carrier lanes) / -2*col*255 (low lanes riding a
    carrier unit, whose m is 0 so w = -env/2).  Pure-low units matmul their
    colors (col*255) directly against env.  Colors are pre-scaled by 255 so
    the clamp is one (max 0, min 255) tensor_scalar straight to uint8.
  - Positions are processed in PAIRS per activation (per-act fixed cost
    amortized); squares and merges are batched per group on DVE.  The image
    accumulates in PSUM tiles packing 3 positions at partition bases
    0/32/64 (zero-weight matmuls open/close each tile's accumulation
    group), so one clamp tensor_scalar serves 3 positions.
  - 2 input DMAs; output leaves via 16 small per-position DMAs on the SP
    queue (HWDGE), keeping Pool free.
"""

import math
import numpy as np

try:
    import concourse.bass as bass
except ImportError:
    import sys
    sys.path.insert(0, "/opt/trn_rl_repo")
    import concourse.bass as bass

import ml_dtypes
import concourse.tile as tile
from concourse import bacc, mybir
from concourse.bass_utils import run_bass_kernel_spmd

F32 = mybir.dt.float32
F32R = mybir.dt.float32r
BF16 = mybir.dt.bfloat16
U8 = mybir.dt.uint8
OP = mybir.AluOpType
AF = mybir.ActivationFunctionType

H = 256
W = 256
G = 4
NCORES = 8
ROWS = H // NCORES          # 32 rows per core
POSW = 16                   # columns per position
NPOS = W // POSW            # 16 positions per core
PPX = POSW * ROWS           # 512 px per position
PX = ROWS * W               # 8192 px per core
INV2PI = 1.0 / (2.0 * math.pi)
EPS = 5e-4                  # contribution cutoff amp*exp(-sigma_min)
BF16NP = ml_dtypes.bfloat16

_CACHE = {}

# Route Sin through the exp_and_friends activation table set: the HW table
# function sin2pi(x) = sin(2*pi*x) is exact for |x| <= 0.5 (measured) and
# lives in the same set as exp, so one table load serves the whole kernel.
# bass's enum has no Sin2pi, so we build with AF.Sin (scale chosen so the
# interpreter computes the same values) and rename the func in the BIR JSON
# at serialization time; insert_act_table_loads is steered by advertising
# Sin as a member of exp_and_friends (and of nothing else).
_TABLES_PATCHED = False


def _patch_act_tables():
    global _TABLES_PATCHED
    if _TABLES_PATCHED:
        return
    orig = bacc.get_activation_tables

    def patched(arch):
        t = orig(arch)
        out = {n: set(fs) for n, fs in t.items()}
        for n in out:
            out[n].discard(AF.Sin)
        out["exp_and_friends"].add(AF.Sin)
        return out

    bacc.get_activation_tables = patched
    _TABLES_PATCHED = True


# engine allocation knobs (sim-swept): merge/square engine per unit index:
# 'V' = DVE, 'P' = Pool, 'A' = alternate by unit; clamp_pair: one clamp per
# activation pair (else per position); quad_a: 4-position Sin batches.
CFG = {"merge": "V", "square": "V", "clamp_pair": False, "quad_a": False,
       "bsingle": False, "sigbufs": 2, "imgbufs": 2, "sqbatch": True,
       "noclose": True}

# basis column offsets in pa: [basis16 | basisq16 | basis8 | basisq8 | lhsT..]
_B16, _BQ16, _B8, _BQ8, _BEND = 0, 512, 1024, 1280, 1536

# key: tuple over positions of unit tuples (pxw, has_car, seg_idx).
# Units are in segment order (left-to-right), chunks of a segment adjacent.
# The output pixel layout of a position is its segments' pixels in order,
# row-major within each segment; multi-chunk segments appear once.


def _seg_widths(posunits):
    seen = {}
    for (pxw, hc, sidx) in posunits:
        seen[sidx] = pxw
    return [seen[s] for s in sorted(seen)]


def _build_program(key, repeat=1, hwpatch=True):
    _patch_act_tables()
    nc = bacc.Bacc("TRN2", target_bir_lowering=False, debug=False)
    units = key
    tot_units = sum(len(u) for u in units)
    tot_car = sum(1 for u in units for (_, hc, _) in u if hc)
    CA = _BEND + 128 * tot_units + 128 * tot_car

    # column offsets (must match _host_inputs layout)
    sig_off = {}
    ph_off = {}
    slot_off = {}
    pb_off = {}
    nrow_t = {t: 32 * (min(3, NPOS - 3 * t) - 1) + 3
              for t in range((NPOS + 2) // 3)}
    tri_units = {t: 0 for t in nrow_t}
    c = _BEND
    cb = 0
    dimg = CFG.get("dimg", False)
    for p in range(NPOS):
        for j, (pxw, hc, sidx) in enumerate(units[p]):
            sig_off[(p, j)] = c
            c += 128
            pb_off[(p, j)] = cb
            cb += 6 if (dimg and hc) else 3
            tri_units[p // 3] += 1
    # pb: color cols per unit, padded to >=512 (zero-matmul rhs), plus a
    # 67-col zero block for the per-triple psum initializer lhsT
    CB = max(cb, 512) + 67
    pa = nc.declare_dram_parameter("pa", [12, CA], F32, isOutput=False)
    ZOFF = CB - 67
    pb = nc.declare_dram_parameter("pb", [128, CB], BF16, isOutput=False)
    out_ext = nc.declare_dram_parameter("out", [3, PX], U8, isOutput=True)
    scol = 0
    for p in range(NPOS):
        for j, (pxw, hc, sidx) in enumerate(units[p]):
            if hc:
                ph_off[(p, j)] = c
                c += 128
                slot_off[(p, j)] = scol
                scol += pxw
    assert c == CA
    mcols = max(1, scol)
    # env-psum width per position (units side by side, chunks duplicated)
    envw = {p: sum(pxw for (pxw, _, _) in units[p]) for p in range(NPOS)}
    carw = {p: sum(pxw for (pxw, hc, _) in units[p] if hc)
            for p in range(NPOS)}
    # geometric pixel offset of each segment inside the position's 512 px
    seg_px_off = {}
    for p in range(NPOS):
        o = 0
        for s, w in enumerate(_seg_widths(units[p])):
            seg_px_off[(p, s)] = o
            o += w
        assert o == PPX

    V = nc.vector
    S = nc.scalar
    T = nc.tensor
    PI = math.pi

    # position pairs for act batching (psum width cap 1024 f32 cols)
    pairs = []
    p = 0
    while p < NPOS:
        if p + 1 < NPOS and envw[p] + envw[p + 1] <= 1024:
            pairs.append((p, p + 1))
            p += 2
        else:
            pairs.append((p,))
            p += 1
    # pass-A groups: up to 4 positions per Sin (psum cap 2048 f32 cols)
    agroups = []
    p = 0
    while p < NPOS:
        g = [p]
        w = carw[p]
        while p + len(g) < NPOS and w + carw[p + len(g)] <= 2048 \
                and len(g) < 4:
            w += carw[p + len(g)]
            g.append(p + len(g))
        agroups.append(tuple(g))
        p += len(g)

    with tile.TileContext(nc, pool_alloc_mode="queue") as tc:
        with tc.tile_pool(name="singles", bufs=1) as singles:
            pa_sb = singles.tile([12, CA], F32R)
            nc.gpsimd.dma_start(out=pa_sb, in_=pa[:])
            pb_sb = singles.tile([128, CB], BF16)
            nc.sync.dma_start(out=pb_sb, in_=pb[:])
            outu67 = singles.tile([67, (NPOS + 2) // 3 * PPX], U8)
            mbuf = singles.tile([128, mcols], BF16)

            def bq(pxw):
                return (pa_sb[0:6, _BQ16:_BQ16 + 512] if pxw == 512
                        else pa_sb[0:6, _BQ8:_BQ8 + 256])

            def bs(pxw):
                return (pa_sb[:, _B16:_B16 + 512] if pxw == 512
                        else pa_sb[:, _B8:_B8 + 256])

            for _rep in range(repeat):
                # ---- pass A: carriers (Sin table): phase -> sin^2 ----
                with tc.tile_pool(name="ph", bufs=2, space="PSUM") as php, \
                     tc.tile_pool(name="sgp", bufs=2) as sgp:
                    for pr in (agroups if CFG["quad_a"] else pairs):
                        cw = sum(carw[p] for p in pr)
                        if cw == 0:
                            continue
                        ph = php.tile([128, cw], F32, name="ph", tag="ph")
                        sg = sgp.tile([128, cw], BF16, name="sg", tag="sg")
                        o = 0
                        offs = []
                        for p in pr:
                            for j, (pxw, hc, sidx) in enumerate(units[p]):
                                if not hc:
                                    continue
                                po = ph_off[(p, j)]
                                T.matmul(ph[:, o:o + pxw],
                                         pa_sb[0:6, po:po + 128], bq(pxw),
                                         start=True, stop=True)
                                offs.append((p, j, o, pxw))
                                o += pxw
                        S.activation(sg, ph, AF.Sin,
                                     scale=(1.0 if hwpatch else 2.0 * PI))
                        gi = pr[0]
                        sqE = {"V": V, "P": nc.gpsimd}.get(
                            CFG["square"], V if (gi // 2) % 2 else nc.gpsimd)
                        if CFG.get("sqbatch") and offs:
                            # group's mbuf slots are contiguous
                            so0 = slot_off[(offs[0][0], offs[0][1])]
                            sqE.tensor_tensor(out=mbuf[:, so0:so0 + cw],
                                              in0=sg, in1=sg, op=OP.mult)
                        else:
                            for ui, (p, j, o, pxw) in enumerate(offs):
                                so = slot_off[(p, j)]
                                sqE.tensor_tensor(out=mbuf[:, so:so + pxw],
                                                  in0=sg[:, o:o + pxw],
                                                  in1=sg[:, o:o + pxw],
                                                  op=OP.mult)

                # ---- pass B: envelopes + image (Exp table) ----
                with tc.tile_pool(name="sig", bufs=CFG["sigbufs"],
                                  space="PSUM") as sigp, \
                     tc.tile_pool(name="envp", bufs=3) as envp, \
                     tc.tile_pool(name="wp", bufs=4) as wp, \
                     tc.tile_pool(name="imgp", bufs=CFG["imgbufs"],
                                  space="PSUM") as imgp:
                    bgroups = ([(p,) for p in range(NPOS)]
                               if CFG["bsingle"] else pairs)

                    def do_sig(pr):
                        tw = sum(envw[p] for p in pr)
                        sig = sigp.tile([128, tw], F32, name="sig", tag="sig")
                        o = 0
                        offs = []
                        for p in pr:
                            for j, (pxw, hc, sidx) in enumerate(units[p]):
                                so = sig_off[(p, j)]
                                T.matmul(sig[:, o:o + pxw],
                                         pa_sb[:, so:so + 128], bs(pxw),
                                         start=True, stop=True)
                                offs.append((p, j, o, pxw, hc, sidx))
                                o += pxw
                        return sig, offs, tw

                    def do_rest(pr, sig, offs, tw):
                        env = envp.tile([128, tw], BF16, name="env",
                                        tag="env")
                        S.activation(env, sig, AF.Exp, scale=-1.0)
                        # img psum packs 3 positions per tile at PE-legal
                        # partition bases 0/32/64; the triple's first matmul
                        # uses a wide zero-padded color lhsT (start=True) so
                        # every row is initialized; one clamp covers all 3
                        for p in pr:
                            t = p // 3
                            if t not in trip:
                                tl = imgp.tile([nrow_t[t], PPX], F32,
                                               name="img", tag="img")
                                # zero-weight matmul opens the tile's psum
                                # accumulation group and zeroes every row
                                T.matmul(tl[0:nrow_t[t], :],
                                         pb_sb[:, ZOFF:ZOFF + nrow_t[t]],
                                         pb_sb[:, 0:PPX],
                                         start=True, stop=False)
                                trip[t] = [tl, 0]
                        # batched merge: when every unit in the group has
                        # carriers (the common case) the group's mbuf slots
                        # and env columns line up, so one op covers them all
                        allcar = all(hc for (_, _, _, _, hc, _) in offs)
                        mgE = {"V": V, "P": nc.gpsimd}.get(
                            CFG["merge"],
                            V if (pr[0] // 2) % 2 else nc.gpsimd)
                        wslices = {}
                        if allcar:
                            wt = wp.tile([128, tw], BF16, name="w", tag="w")
                            so0 = slot_off[(offs[0][0], offs[0][1])]
                            if dimg:
                                mgE.tensor_tensor(
                                    out=wt, in0=mbuf[:, so0:so0 + tw],
                                    in1=env, op=OP.mult)
                            else:
                                mgE.scalar_tensor_tensor(
                                    out=wt, in0=mbuf[:, so0:so0 + tw],
                                    scalar=-0.5, in1=env,
                                    op0=OP.add, op1=OP.mult)
                            for (p, j, o, pxw, hc, sidx) in offs:
                                wslices[(p, j)] = wt[:, o:o + pxw]
                        else:
                            for (p, j, o, pxw, hc, sidx) in offs:
                                if hc:
                                    so = slot_off[(p, j)]
                                    w = wp.tile([128, pxw], BF16, name="w",
                                                tag="w")
                                    if dimg:
                                        mgE.tensor_tensor(
                                            out=w,
                                            in0=mbuf[:, so:so + pxw],
                                            in1=env[:, o:o + pxw],
                                            op=OP.mult)
                                    else:
                                        mgE.scalar_tensor_tensor(
                                            out=w, in0=mbuf[:, so:so + pxw],
                                            scalar=-0.5,
                                            in1=env[:, o:o + pxw],
                                            op0=OP.add, op1=OP.mult)
                                    wslices[(p, j)] = w
                                else:
                                    wslices[(p, j)] = env[:, o:o + pxw]
                        for (p, j, o, pxw, hc, sidx) in offs:
                            t, s = divmod(p, 3)
                            tile3, nm = trip[t]
                            trip[t][1] = nm + 1
                            xo = seg_px_off[(p, sidx)]
                            bo = pb_off[(p, j)]
                            lastmm = (CFG.get("noclose")
                                      and trip[t][1] == tri_units[t])
                            if dimg and hc:
                                T.matmul(
                                    tile3[32 * s:32 * s + 3, xo:xo + pxw],
                                    pb_sb[:, bo:bo + 3], env[:, o:o + pxw],
                                    start=False, stop=False)
                                T.matmul(
                                    tile3[32 * s:32 * s + 3, xo:xo + pxw],
                                    pb_sb[:, bo + 3:bo + 6],
                                    wslices[(p, j)],
                                    start=False, stop=lastmm)
                            else:
                                T.matmul(
                                    tile3[32 * s:32 * s + 3, xo:xo + pxw],
                                    pb_sb[:, bo:bo + 3], wslices[(p, j)],
                                    start=False, stop=lastmm)
                        for p in pr:
                            t = p // 3
                            if t in trip and trip[t][1] == tri_units[t]:
                                tile3, _ = trip.pop(t)
                                nrow = nrow_t[t]
                                if not CFG.get("noclose"):
                                    # close the tile's accumulation group
                                    T.matmul(tile3[0:nrow, :],
                                             pb_sb[:, ZOFF:ZOFF + nrow],
                                             pb_sb[:, 0:PPX],
                                             start=False, stop=True)
                                V.tensor_scalar(
                                    outu67[0:nrow, t * PPX:(t + 1) * PPX],
                                    tile3, 0.0, 255.0, OP.max, OP.min)
                                for ss in range(min(3, NPOS - 3 * t)):
                                    nc.sync.dma_start(
                                        out=out_ext[:, (3 * t + ss) * PPX:
                                                    (3 * t + ss + 1) * PPX],
                                        in_=outu67[32 * ss:32 * ss + 3,
                                                   t * PPX:(t + 1) * PPX])

                    trip = {}
                    prev = None
                    for pr in bgroups:
                        cur = (pr,) + do_sig(pr)
                        if prev is not None:
                            do_rest(*prev)
                        prev = cur
                    do_rest(*prev)
    nc.finalize()
    if hwpatch:
        patched = nc.to_json_bytes().replace(b'"func":"Sin"',
                                             b'"func":"Sin2pi"')
        nc.to_json_bytes = lambda: patched
    return nc


def _planes(mu, chol, feat, opac):
    """Global per-gaussian conic + color, f64."""
    mu = np.asarray(mu, np.float64)
    chol = np.asarray(chol, np.float64)
    m = np.tanh(mu)
    xa = (m[:, 0] + 1.0) * 0.5 * W
    ya = (m[:, 1] + 1.0) * 0.5 * H
    l1 = chol[:, 0] + 0.5
    l2 = chol[:, 1]
    l3 = chol[:, 2] + 0.5
    sxx = l1 * l1
    sxy = l1 * l2
    syy = l2 * l2 + l3 * l3
    det = sxx * syy - sxy * sxy
    A = syy / det
    B = -sxy / det
    C = sxx / det
    colf = np.asarray(feat, np.float64) * np.asarray(opac, np.float64)
    return xa, ya, A, B, C, colf


def _bf16r(w):
    return np.asarray(w, np.float32).astype(BF16NP).astype(np.float64)


def _sigma_min(A, B, C, xa, ya, x0, x1, y0, y1):
    cx = np.clip(xa, x0, x1)
    cy = np.clip(ya, y0, y1)
    dx = cx - xa
    dy = cy - ya
    return 0.5 * A * dx * dx + B * dx * dy + 0.5 * C * dy * dy


def _mk_basis(w):
    rr, ccc = np.meshgrid(np.arange(ROWS), np.arange(w), indexing="ij")
    xc = (ccc - (w - 1) / 2.0).ravel()
    yc = (rr - 15.5).ravel()
    b = np.zeros((12, w * ROWS))
    b[0] = b[1] = xc * xc
    b[2] = b[3] = xc * yc
    b[4] = b[5] = yc * yc
    b[6] = b[7] = xc
    b[8] = b[9] = yc
    b[10] = b[11] = 1.0
    q = np.zeros((12, w * ROWS))
    q[0] = xc
    q[1] = yc
    xh = (xc >= 0).astype(np.float64)
    yh = (yc >= 0).astype(np.float64)
    for qq in range(4):
        qx, qy = divmod(qq, 2)
        q[2 + qq] = ((xh == qx) & (yh == qy)).astype(np.float64)
    qcx = np.array([-w / 4.0, -w / 4.0, w / 4.0, w / 4.0])
    qcy = np.array([-8.0, 8.0, -8.0, 8.0])
    return b, q, qcx, qcy


def _host_inputs(low_mu, high_mu, low_chol, high_chol, low_feat, high_feat,
                 low_opac, high_opac, gabor_freqs, gabor_weights):
    xaL, yaL, AL, BL, CL, colL = _planes(low_mu, low_chol, low_feat, low_opac)
    xaH, yaH, AH, BH, CH, colH = _planes(high_mu, high_chol, high_feat,
                                         high_opac)
    gf = np.asarray(gabor_freqs, np.float64)
    gw = np.asarray(gabor_weights, np.float64)
    nH = xaH.shape[0]
    fx = gf[:, 0].reshape(nH, G) * INV2PI   # cycles / px
    fy = gf[:, 1].reshape(nH, G) * INV2PI
    wg = gw[:, 0].reshape(nH, G)
    ampL = np.abs(colL).max(1)
    ampH = np.abs(colH).max(1) * np.abs(wg).sum(1)

    def select(k, x0, x1):
        y0, y1 = ROWS * k, ROWS * (k + 1)
        sL = _sigma_min(AL, BL, CL, xaL, yaL, x0, x1, y0, y1)
        sH = _sigma_min(AH, BH, CH, xaH, yaH, x0, x1, y0, y1)
        iL = np.nonzero(ampL * np.exp(-sL) >= EPS)[0]
        iH = np.nonzero(ampH * np.exp(-sH) >= EPS)[0]
        return iL, iH

    def lanecount(sel):
        return len(sel[0]) + 4 * len(sel[1])

    # ---- shared program geometry ----
    segs_of_pos = []   # per position: list of (x0, x1, pxw, nchunk, ncar)
    units = []
    for p in range(NPOS):
        s16 = [select(k, POSW * p, POSW * (p + 1)) for k in range(NCORES)]
        if max(lanecount(s) for s in s16) <= 128:
            seglist = [(POSW * p, POSW * (p + 1), 512, s16)]
        else:
            seglist = [
                (POSW * p, POSW * p + 8, 256,
                 [select(k, POSW * p, POSW * p + 8) for k in range(NCORES)]),
                (POSW * p + 8, POSW * (p + 1), 256,
                 [select(k, POSW * p + 8, POSW * (p + 1))
                  for k in range(NCORES)])]
        segs = []
        ulist = []
        for sidx, (x0, x1, pxw, sels) in enumerate(seglist):
            nchunk = max(max(1, -(-lanecount(s) // 128)) for s in sels)
            ncar = max(-(-(4 * len(s[1])) // 128) for s in sels)
            segs.append((x0, x1, pxw, nchunk, ncar))
            for j in range(nchunk):
                ulist.append((pxw, j < ncar, sidx))
        segs_of_pos.append(segs)
        units.append(tuple(ulist))
    key = tuple(units)

    tot_units = sum(len(u) for u in units)
    CA = _BEND + 128 * tot_units \
        + 128 * sum(1 for u in units for (_, hc, _) in u if hc)
    pb_off_h = {}
    cbh = 0
    for p in range(NPOS):
        for j, (pxw, hc, sidx) in enumerate(units[p]):
            pb_off_h[(p, j)] = cbh
            cbh += 6 if (CFG.get("dimg", False) and hc) else 3
    CB = max(cbh, 512) + 67

    b16, q16, qcx16, qcy16 = _mk_basis(16)
    b8, q8, qcx8, qcy8 = _mk_basis(8)

    in_maps = []
    for k in range(NCORES):
        pa = np.zeros((12, CA), np.float64)
        pbv = np.zeros((128, CB), np.float64)
        pa[:, _B16:_B16 + 512] = b16
        pa[:, _BQ16:_BQ16 + 512] = q16
        pa[:, _B8:_B8 + 256] = b8
        pa[:, _BQ8:_BQ8 + 256] = q8
        Yc = ROWS * k + 16.0
        # column cursors mirror _build_program's offset assignment
        c_sig = _BEND
        c_ph = _BEND + 128 * tot_units
        ci = 0
        for p in range(NPOS):
            ju = 0
            for (x0, x1, pxw, nchunk, ncar) in segs_of_pos[p]:
                iL, iH = select(k, x0, x1)
                nLa, nHa = len(iL), len(iH)
                nl = 4 * nHa + nLa
                Xc = (x0 + x1) / 2.0
                qcx, qcy = (qcx16, qcy16) if pxw == 512 else (qcx8, qcy8)
                # vectorized per-lane quantities, carrier lanes first
                isc = np.zeros(nl, bool)
                isc[:4 * nHa] = True
                hrep = np.repeat(iH, G)
                A = np.concatenate([AH[hrep], AL[iL]])
                B = np.concatenate([BH[hrep], BL[iL]])
                C = np.concatenate([CH[hrep], CL[iL]])
                ax = Xc - np.concatenate([xaH[hrep], xaL[iL]])
                ay = Yc - np.concatenate([yaH[hrep], yaL[iL]])
                ws = np.stack([0.5 * A, B, 0.5 * C,
                               A * ax + B * ay, B * ax + C * ay,
                               0.5 * A * ax * ax + B * ax * ay
                               + 0.5 * C * ay * ay])           # [6, nl]
                hi = _bf16r(ws)
                sig12 = np.zeros((12, nl))
                sig12[0::2] = hi
                sig12[1::2] = ws - hi
                gl = np.arange(4 * nHa) % G
                fxc = fx[hrep, gl]
                fyc = fy[hrep, gl]
                base = fxc * ax[:4 * nHa] + fyc * ay[:4 * nHa]
                ph6 = np.zeros((6, 4 * nHa))
                ph6[0] = fxc
                ph6[1] = fyc
                for qq in range(4):
                    ph6[2 + qq] = base - np.round(base + fxc * qcx[qq]
                                                  + fyc * qcy[qq])
                ph6 *= 0.5   # half angle in cycle units for sin2pi
                if CFG.get("dimg", False):
                    colsE = np.concatenate([
                        colH[hrep] * wg[hrep, gl][:, None] * 255.0,
                        colL[iL] * 255.0])
                    colsM = np.concatenate([
                        -2.0 * colH[hrep] * wg[hrep, gl][:, None] * 255.0,
                        np.zeros((nLa, 3))])
                else:
                    cols = np.concatenate([
                        -2.0 * colH[hrep] * wg[hrep, gl][:, None] * 255.0,
                        colL[iL] * 255.0])                     # [nl, 3]
                for j in range(nchunk):
                    lo, hi_ = j * 128, min((j + 1) * 128, nl)
                    n = max(0, hi_ - lo)
                    so = c_sig
                    c_sig += 128
                    has_car = j < ncar
                    pa[10, so:so + 128] = 1000.0   # pad: sigma -> env 0
                    if n > 0:
                        pa[:, so:so + n] = sig12[:, lo:hi_]
                    if has_car:
                        po = c_ph
                        c_ph += 128
                        ncl = max(0, min(hi_, 4 * nHa) - lo)
                        if ncl > 0:
                            pa[0:6, po:po + ncl] = ph6[:, lo:lo + ncl]
                        # L lanes in this unit ride w=-env/2: color -2*col*255
                        if not CFG.get("dimg", False) and n > ncl:
                            cols[lo + ncl:hi_] *= -2.0
                    if n > 0:
                        bo = pb_off_h[(p, ju)]
                        if CFG.get("dimg", False) and has_car:
                            pbv[0:n, bo:bo + 3] = colsE[lo:hi_]
                            pbv[0:n, bo + 3:bo + 6] = colsM[lo:hi_]
                        elif CFG.get("dimg", False):
                            pbv[0:n, bo:bo + 3] = colsE[lo:hi_]
                        else:
                            pbv[0:n, bo:bo + 3] = cols[lo:hi_]
                    ci += 1
                    ju += 1
        in_maps.append({
            "pa": np.ascontiguousarray(pa, np.float32),
            "pb": np.ascontiguousarray(pbv.astype(np.float32)
                                       .astype(BF16NP)),
        })
    return in_maps, key


_GMAP = {}


def _gmap_from_key(key):
    """colmap[r, c] = output column holding geometric pixel (r, c)."""
    if key in _GMAP:
        return _GMAP[key]
    colmap = np.zeros((ROWS, W), np.int64)
    for p in range(NPOS):
        base = p * PPX
        off = 0
        for w in _seg_widths(key[p]):
            wcols = w // ROWS
            x0 = POSW * p + (off // ROWS)
            rr, ccc = np.meshgrid(np.arange(ROWS), np.arange(x0, x0 + wcols),
                                  indexing="ij")
            colmap[rr, ccc] = base + off + (rr * wcols + (ccc - x0))
            off += w
    _GMAP[key] = colmap.reshape(-1)
    return _GMAP[key]


def _assemble(results, key):
    gmap = _gmap_from_key(key)
    img = np.zeros((3, H, W), np.float32)
    for k in range(NCORES):
        o = np.asarray(results[k]["out"]).astype(np.float32) / 255.0
        img[:, k * ROWS:(k + 1) * ROWS, :] = o[:, gmap].reshape(3, ROWS, W)
    return img[None]


def kernel(**inputs):
    inputs = {k: np.asarray(v, np.float32) for k, v in inputs.items()}
    in_maps, key = _host_inputs(**inputs)
    if key not in _CACHE:
        _CACHE[key] = _build_program(key)
    nc = _CACHE[key]
    res = run_bass_kernel_spmd(nc, in_maps, list(range(NCORES)))
    return _assemble(res.results, key).astype(np.float32)


if __name__ == "__main__":
    import reference
    ins = {k: np.asarray(v) for k, v in reference.setup_inputs().items()}
    out = kernel(**ins)
    ref = np.asarray(reference.reference(**reference.setup_inputs()))
    rel = np.linalg.norm(out - ref) / np.linalg.norm(ref)
    print("Relative error:", rel)


# revision 36
# speedup vs baseline: 1.6092x; 1.6092x over previous
"""Trainium2 Bass kernel for mixed Gaussian/Gabor splat rasterization.

Problem: render 3072 plain 2D gaussians + 1024 gabor-modulated gaussians
(G=4 cosine carriers each) densely into a [1,3,256,256] image, clamp [0,1].

v3 design (carrier-major, host-heavy, adaptive blocks):
  - Pixels sharded across 8 cores: core k owns rows [32k, 32k+32), split into
    16 column POSITIONS of 16x32 px.  A position whose densest core has more
    than 128 overlapping lanes is split into two 8x32 SEGMENTS, so (for these
    inputs) every segment's lanes fit one 128-partition chunk and no pixel is
    rasterized twice.  Every engine op's cost scales with its free size
    (pixels), not partitions, so this halves the dense positions' work.
  - Lanes: a low gaussian is 1 lane; a high gaussian is 4 carrier lanes (one
    per gabor carrier), each carrying its own copy of the parent's sigma
    rows.  Selection is contribution-based: keep a gaussian in a segment if
    amp * exp(-sigma(closest point)) >= EPS, where amp is |color|max (low) or
    |color|max * sum|wg| (high).
  - All matmul lhsT data is precomputed on the host in f64:
      sigma [12,128]: bf16 hi/lo split weights [w0h,w0l..w5h,w5l] against
        basis rows [x2,x2,xy,xy,y2,y2,x,x,y,y,1,1] (basis values are exact in
        f32r), with the constant w5 folded in so Exp needs no bias.
      phase [6,128]: [fx,fy,o0..o3] half-angle cycles/px + per-quarter
        rounded offsets (one-hot quarter rows in the pixel basis) keep the
        activation argument within +-0.38 cycles.  The Sin activation is
        renamed to the HW table function Sin2pi (= sin(2*pi*x), measured
        exact for |x| <= 0.5) in the BIR JSON at serialization time;
        Sin2pi lives in the same act-table set as Exp ("exp_and_friends"),
        so the whole kernel needs ONE table load instead of two per pass.
  - cos via half angle with the -2 folded into the colors: pass A computes
    m = sin^2(pi t) (Sin -> bf16, square on DVE in 2x mode); pass B merges
    w = (m - 0.5) * env, and a carrier unit's single color matmul uses
    c = -2*col*wg*255 (carrier lanes) / -2*col*255 (low lanes riding a
    carrier unit, whose m is 0 so w = -env/2).  Pure-low units matmul their
    colors (col*255) directly against env.  Colors are pre-scaled by 255 so
    the clamp is one (max 0, min 255) tensor_scalar straight to uint8.
  - Positions are processed in PAIRS per activation (per-act fixed cost
    amortized); squares and merges are batched per group on DVE.  The image
    accumulates in PSUM tiles packing 3 positions at partition bases
    0/32/64 (zero-weight matmuls open/close each tile's accumulation
    group), so one clamp tensor_scalar serves 3 positions.
  - 2 input DMAs; output leaves via 16 small per-position DMAs on the SP
    queue (HWDGE), keeping Pool free.
"""

import math
import numpy as np

try:
    import concourse.bass as bass
except ImportError:
    import sys
    sys.path.insert(0, "/opt/trn_rl_repo")
    import concourse.bass as bass

import ml_dtypes
import concourse.tile as tile
from concourse import bacc, mybir
from concourse.bass_utils import run_bass_kernel_spmd

F32 = mybir.dt.float32
F32R = mybir.dt.float32r
BF16 = mybir.dt.bfloat16
U8 = mybir.dt.uint8
OP = mybir.AluOpType
AF = mybir.ActivationFunctionType

H = 256
W = 256
G = 4
NCORES = 8
ROWS = H // NCORES          # 32 rows per core
POSW = 16                   # columns per position
NPOS = W // POSW            # 16 positions per core
PPX = POSW * ROWS           # 512 px per position
PX = ROWS * W               # 8192 px per core
INV2PI = 1.0 / (2.0 * math.pi)
EPS = 5e-4                  # contribution cutoff amp*exp(-sigma_min)
BF16NP = ml_dtypes.bfloat16

_CACHE = {}

# Route Sin through the exp_and_friends activation table set: the HW table
# function sin2pi(x) = sin(2*pi*x) is exact for |x| <= 0.5 (measured) and
# lives in the same set as exp, so one table load serves the whole kernel.
# bass's enum has no Sin2pi, so we build with AF.Sin (scale chosen so the
# interpreter computes the same values) and rename the func in the BIR JSON
# at serialization time; insert_act_table_loads is steered by advertising
# Sin as a member of exp_and_friends (and of nothing else).
_TABLES_PATCHED = False


def _patch_act_tables():
    global _TABLES_PATCHED
    if _TABLES_PATCHED:
        return
    orig = bacc.get_activation_tables

    def patched(arch):
        t = orig(arch)
        out = {n: set(fs) for n, fs in t.items()}
        for n in out:
            out[n].discard(AF.Sin)
        out["exp_and_friends"].add(AF.Sin)
        return out

    bacc.get_activation_tables = patched
    _TABLES_PATCHED = True


# engine allocation knobs (sim-swept): merge/square engine per unit index:
# 'V' = DVE, 'P' = Pool, 'A' = alternate by unit; clamp_pair: one clamp per
# activation pair (else per position); quad_a: 4-position Sin batches.
CFG = {"merge": "V", "square": "V", "clamp_pair": False, "quad_a": False,
       "bsingle": False, "sigbufs": 2, "imgbufs": 2, "sqbatch": True,
       "noclose": True, "phbufs": 2, "sgbufs": 4, "envbufs": 5, "wbufs": 5}

# basis column offsets in pa: [basis16 | basisq16 | basis8 | basisq8 | lhsT..]
_B16, _BQ16, _B8, _BQ8, _BEND = 0, 512, 1024, 1280, 1536

# key: tuple over positions of unit tuples (pxw, has_car, seg_idx).
# Units are in segment order (left-to-right), chunks of a segment adjacent.
# The output pixel layout of a position is its segments' pixels in order,
# row-major within each segment; multi-chunk segments appear once.


def _seg_widths(posunits):
    seen = {}
    for (pxw, hc, sidx) in posunits:
        seen[sidx] = pxw
    return [seen[s] for s in sorted(seen)]


def _build_program(key, repeat=1, hwpatch=True):
    _patch_act_tables()
    nc = bacc.Bacc("TRN2", target_bir_lowering=False, debug=False)
    units = key
    tot_units = sum(len(u) for u in units)
    tot_car = sum(1 for u in units for (_, hc, _) in u if hc)
    CA = _BEND + 128 * tot_units + 128 * tot_car

    # column offsets (must match _host_inputs layout)
    sig_off = {}
    ph_off = {}
    slot_off = {}
    pb_off = {}
    nrow_t = {t: 32 * (min(3, NPOS - 3 * t) - 1) + 3
              for t in range((NPOS + 2) // 3)}
    tri_units = {t: 0 for t in nrow_t}
    c = _BEND
    cb = 0
    dimg = CFG.get("dimg", False)
    for p in range(NPOS):
        for j, (pxw, hc, sidx) in enumerate(units[p]):
            sig_off[(p, j)] = c
            c += 128
            pb_off[(p, j)] = cb
            cb += 6 if (dimg and hc) else 3
            tri_units[p // 3] += 1
    # pb: color cols per unit, padded to >=512 (zero-matmul rhs), plus a
    # 67-col zero block for the per-triple psum initializer lhsT
    CB = max(cb, 512) + 67
    pa = nc.declare_dram_parameter("pa", [12, CA], F32, isOutput=False)
    ZOFF = CB - 67
    pb = nc.declare_dram_parameter("pb", [128, CB], BF16, isOutput=False)
    out_ext = nc.declare_dram_parameter("out", [3, PX], U8, isOutput=True)
    scol = 0
    for p in range(NPOS):
        for j, (pxw, hc, sidx) in enumerate(units[p]):
            if hc:
                ph_off[(p, j)] = c
                c += 128
                slot_off[(p, j)] = scol
                scol += pxw
    assert c == CA
    mcols = max(1, scol)
    # env-psum width per position (units side by side, chunks duplicated)
    envw = {p: sum(pxw for (pxw, _, _) in units[p]) for p in range(NPOS)}
    carw = {p: sum(pxw for (pxw, hc, _) in units[p] if hc)
            for p in range(NPOS)}
    # geometric pixel offset of each segment inside the position's 512 px
    seg_px_off = {}
    for p in range(NPOS):
        o = 0
        for s, w in enumerate(_seg_widths(units[p])):
            seg_px_off[(p, s)] = o
            o += w
        assert o == PPX

    V = nc.vector
    S = nc.scalar
    T = nc.tensor
    PI = math.pi

    # position pairs for act batching (psum width cap 1024 f32 cols)
    pairs = []
    p = 0
    while p < NPOS:
        if p + 1 < NPOS and envw[p] + envw[p + 1] <= 1024:
            pairs.append((p, p + 1))
            p += 2
        else:
            pairs.append((p,))
            p += 1
    # pass-A groups: up to 4 positions per Sin (psum cap 2048 f32 cols)
    agroups = []
    p = 0
    while p < NPOS:
        g = [p]
        w = carw[p]
        while p + len(g) < NPOS and w + carw[p + len(g)] <= 2048 \
                and len(g) < 4:
            w += carw[p + len(g)]
            g.append(p + len(g))
        agroups.append(tuple(g))
        p += len(g)

    with tile.TileContext(nc, pool_alloc_mode="queue") as tc:
        with tc.tile_pool(name="singles", bufs=1) as singles:
            pa_sb = singles.tile([12, CA], F32R)
            nc.gpsimd.dma_start(out=pa_sb, in_=pa[:])
            pb_sb = singles.tile([128, CB], BF16)
            nc.sync.dma_start(out=pb_sb, in_=pb[:])
            outu67 = singles.tile([67, (NPOS + 2) // 3 * PPX], U8)
            mbuf = singles.tile([128, mcols], BF16)

            def bq(pxw):
                return (pa_sb[0:6, _BQ16:_BQ16 + 512] if pxw == 512
                        else pa_sb[0:6, _BQ8:_BQ8 + 256])

            def bs(pxw):
                return (pa_sb[:, _B16:_B16 + 512] if pxw == 512
                        else pa_sb[:, _B8:_B8 + 256])

            for _rep in range(repeat):
                # ---- pass A: carriers (Sin table): phase -> sin^2 ----
                with tc.tile_pool(name="ph", bufs=CFG.get("phbufs", 2),
                                  space="PSUM") as php, \
                     tc.tile_pool(name="sgp",
                                  bufs=CFG.get("sgbufs", 2)) as sgp:
                    for pr in (agroups if CFG["quad_a"] else pairs):
                        cw = sum(carw[p] for p in pr)
                        if cw == 0:
                            continue
                        ph = php.tile([128, cw], F32, name="ph", tag="ph")
                        sg = sgp.tile([128, cw], BF16, name="sg", tag="sg")
                        o = 0
                        offs = []
                        for p in pr:
                            for j, (pxw, hc, sidx) in enumerate(units[p]):
                                if not hc:
                                    continue
                                po = ph_off[(p, j)]
                                T.matmul(ph[:, o:o + pxw],
                                         pa_sb[0:6, po:po + 128], bq(pxw),
                                         start=True, stop=True)
                                offs.append((p, j, o, pxw))
                                o += pxw
                        S.activation(sg, ph, AF.Sin,
                                     scale=(1.0 if hwpatch else 2.0 * PI))
                        gi = pr[0]
                        sqE = {"V": V, "P": nc.gpsimd}.get(
                            CFG["square"], V if (gi // 2) % 2 else nc.gpsimd)
                        if CFG.get("sqbatch") and offs:
                            # group's mbuf slots are contiguous
                            so0 = slot_off[(offs[0][0], offs[0][1])]
                            sqE.tensor_tensor(out=mbuf[:, so0:so0 + cw],
                                              in0=sg, in1=sg, op=OP.mult)
                        else:
                            for ui, (p, j, o, pxw) in enumerate(offs):
                                so = slot_off[(p, j)]
                                sqE.tensor_tensor(out=mbuf[:, so:so + pxw],
                                                  in0=sg[:, o:o + pxw],
                                                  in1=sg[:, o:o + pxw],
                                                  op=OP.mult)

                # ---- pass B: envelopes + image (Exp table) ----
                with tc.tile_pool(name="sig", bufs=CFG["sigbufs"],
                                  space="PSUM") as sigp, \
                     tc.tile_pool(name="envp",
                                  bufs=CFG.get("envbufs", 3)) as envp, \
                     tc.tile_pool(name="wp",
                                  bufs=CFG.get("wbufs", 4)) as wp, \
                     tc.tile_pool(name="imgp", bufs=CFG["imgbufs"],
                                  space="PSUM") as imgp:
                    bgroups = ([(p,) for p in range(NPOS)]
                               if CFG["bsingle"] else pairs)

                    def do_sig(pr):
                        tw = sum(envw[p] for p in pr)
                        sig = sigp.tile([128, tw], F32, name="sig", tag="sig")
                        o = 0
                        offs = []
                        for p in pr:
                            for j, (pxw, hc, sidx) in enumerate(units[p]):
                                so = sig_off[(p, j)]
                                T.matmul(sig[:, o:o + pxw],
                                         pa_sb[:, so:so + 128], bs(pxw),
                                         start=True, stop=True)
                                offs.append((p, j, o, pxw, hc, sidx))
                                o += pxw
                        return sig, offs, tw

                    def do_rest(pr, sig, offs, tw):
                        env = envp.tile([128, tw], BF16, name="env",
                                        tag="env")
                        S.activation(env, sig, AF.Exp, scale=-1.0)
                        # img psum packs 3 positions per tile at PE-legal
                        # partition bases 0/32/64; the triple's first matmul
                        # uses a wide zero-padded color lhsT (start=True) so
                        # every row is initialized; one clamp covers all 3
                        for p in pr:
                            t = p // 3
                            if t not in trip:
                                tl = imgp.tile([nrow_t[t], PPX], F32,
                                               name="img", tag="img")
                                # zero-weight matmul opens the tile's psum
                                # accumulation group and zeroes every row
                                T.matmul(tl[0:nrow_t[t], :],
                                         pb_sb[:, ZOFF:ZOFF + nrow_t[t]],
                                         pb_sb[:, 0:PPX],
                                         start=True, stop=False)
                                trip[t] = [tl, 0]
                        # batched merge: when every unit in the group has
                        # carriers (the common case) the group's mbuf slots
                        # and env columns line up, so one op covers them all
                        allcar = all(hc for (_, _, _, _, hc, _) in offs)
                        mgE = {"V": V, "P": nc.gpsimd}.get(
                            CFG["merge"],
                            V if (pr[0] // 2) % 2 else nc.gpsimd)
                        wslices = {}
                        if allcar:
                            wt = wp.tile([128, tw], BF16, name="w", tag="w")
                            so0 = slot_off[(offs[0][0], offs[0][1])]
                            if dimg:
                                mgE.tensor_tensor(
                                    out=wt, in0=mbuf[:, so0:so0 + tw],
                                    in1=env, op=OP.mult)
                            else:
                                mgE.scalar_tensor_tensor(
                                    out=wt, in0=mbuf[:, so0:so0 + tw],
                                    scalar=-0.5, in1=env,
                                    op0=OP.add, op1=OP.mult)
                            for (p, j, o, pxw, hc, sidx) in offs:
                                wslices[(p, j)] = wt[:, o:o + pxw]
                        else:
                            for (p, j, o, pxw, hc, sidx) in offs:
                                if hc:
                                    so = slot_off[(p, j)]
                                    w = wp.tile([128, pxw], BF16, name="w",
                                                tag="w")
                                    if dimg:
                                        mgE.tensor_tensor(
                                            out=w,
                                            in0=mbuf[:, so:so + pxw],
                                            in1=env[:, o:o + pxw],
                                            op=OP.mult)
                                    else:
                                        mgE.scalar_tensor_tensor(
                                            out=w, in0=mbuf[:, so:so + pxw],
                                            scalar=-0.5,
                                            in1=env[:, o:o + pxw],
                                            op0=OP.add, op1=OP.mult)
                                    wslices[(p, j)] = w
                                else:
                                    wslices[(p, j)] = env[:, o:o + pxw]
                        for (p, j, o, pxw, hc, sidx) in offs:
                            t, s = divmod(p, 3)
                            tile3, nm = trip[t]
                            trip[t][1] = nm + 1
                            xo = seg_px_off[(p, sidx)]
                            bo = pb_off[(p, j)]
                            lastmm = (CFG.get("noclose")
                                      and trip[t][1] == tri_units[t])
                            if dimg and hc:
                                T.matmul(
                                    tile3[32 * s:32 * s + 3, xo:xo + pxw],
                                    pb_sb[:, bo:bo + 3], env[:, o:o + pxw],
                                    start=False, stop=False)
                                T.matmul(
                                    tile3[32 * s:32 * s + 3, xo:xo + pxw],
                                    pb_sb[:, bo + 3:bo + 6],
                                    wslices[(p, j)],
                                    start=False, stop=lastmm)
                            else:
                                T.matmul(
                                    tile3[32 * s:32 * s + 3, xo:xo + pxw],
                                    pb_sb[:, bo:bo + 3], wslices[(p, j)],
                                    start=False, stop=lastmm)
                        for p in pr:
                            t = p // 3
                            if t in trip and trip[t][1] == tri_units[t]:
                                tile3, _ = trip.pop(t)
                                nrow = nrow_t[t]
                                if not CFG.get("noclose"):
                                    # close the tile's accumulation group
                                    T.matmul(tile3[0:nrow, :],
                                             pb_sb[:, ZOFF:ZOFF + nrow],
                                             pb_sb[:, 0:PPX],
                                             start=False, stop=True)
                                V.tensor_scalar(
                                    outu67[0:nrow, t * PPX:(t + 1) * PPX],
                                    tile3, 0.0, 255.0, OP.max, OP.min)
                                for ss in range(min(3, NPOS - 3 * t)):
                                    nc.sync.dma_start(
                                        out=out_ext[:, (3 * t + ss) * PPX:
                                                    (3 * t + ss + 1) * PPX],
                                        in_=outu67[32 * ss:32 * ss + 3,
                                                   t * PPX:(t + 1) * PPX])

                    trip = {}
                    prev = None
                    for pr in bgroups:
                        cur = (pr,) + do_sig(pr)
                        if prev is not None:
                            do_rest(*prev)
                        prev = cur
                    do_rest(*prev)
    nc.finalize()
    if hwpatch:
        patched = nc.to_json_bytes().replace(b'"func":"Sin"',
                                             b'"func":"Sin2pi"')
        nc.to_json_bytes = lambda: patched
    return nc


def _planes(mu, chol, feat, opac):
    """Global per-gaussian conic + color, f64."""
    mu = np.asarray(mu, np.float64)
    chol = np.asarray(chol, np.float64)
    m = np.tanh(mu)
    xa = (m[:, 0] + 1.0) * 0.5 * W
    ya = (m[:, 1] + 1.0) * 0.5 * H
    l1 = chol[:, 0] + 0.5
    l2 = chol[:, 1]
    l3 = chol[:, 2] + 0.5
    sxx = l1 * l1
    sxy = l1 * l2
    syy = l2 * l2 + l3 * l3
    det = sxx * syy - sxy * sxy
    A = syy / det
    B = -sxy / det
    C = sxx / det
    colf = np.asarray(feat, np.float64) * np.asarray(opac, np.float64)
    return xa, ya, A, B, C, colf


def _bf16r(w):
    return np.asarray(w, np.float32).astype(BF16NP).astype(np.float64)


def _sigma_min(A, B, C, xa, ya, x0, x1, y0, y1):
    cx = np.clip(xa, x0, x1)
    cy = np.clip(ya, y0, y1)
    dx = cx - xa
    dy = cy - ya
    return 0.5 * A * dx * dx + B * dx * dy + 0.5 * C * dy * dy


def _mk_basis(w):
    rr, ccc = np.meshgrid(np.arange(ROWS), np.arange(w), indexing="ij")
    xc = (ccc - (w - 1) / 2.0).ravel()
    yc = (rr - 15.5).ravel()
    b = np.zeros((12, w * ROWS))
    b[0] = b[1] = xc * xc
    b[2] = b[3] = xc * yc
    b[4] = b[5] = yc * yc
    b[6] = b[7] = xc
    b[8] = b[9] = yc
    b[10] = b[11] = 1.0
    q = np.zeros((12, w * ROWS))
    q[0] = xc
    q[1] = yc
    xh = (xc >= 0).astype(np.float64)
    yh = (yc >= 0).astype(np.float64)
    for qq in range(4):
        qx, qy = divmod(qq, 2)
        q[2 + qq] = ((xh == qx) & (yh == qy)).astype(np.float64)
    qcx = np.array([-w / 4.0, -w / 4.0, w / 4.0, w / 4.0])
    qcy = np.array([-8.0, 8.0, -8.0, 8.0])
    return b, q, qcx, qcy


def _host_inputs(low_mu, high_mu, low_chol, high_chol, low_feat, high_feat,
                 low_opac, high_opac, gabor_freqs, gabor_weights):
    xaL, yaL, AL, BL, CL, colL = _planes(low_mu, low_chol, low_feat, low_opac)
    xaH, yaH, AH, BH, CH, colH = _planes(high_mu, high_chol, high_feat,
                                         high_opac)
    gf = np.asarray(gabor_freqs, np.float64)
    gw = np.asarray(gabor_weights, np.float64)
    nH = xaH.shape[0]
    fx = gf[:, 0].reshape(nH, G) * INV2PI   # cycles / px
    fy = gf[:, 1].reshape(nH, G) * INV2PI
    wg = gw[:, 0].reshape(nH, G)
    ampL = np.abs(colL).max(1)
    ampH = np.abs(colH).max(1) * np.abs(wg).sum(1)

    def select(k, x0, x1):
        y0, y1 = ROWS * k, ROWS * (k + 1)
        sL = _sigma_min(AL, BL, CL, xaL, yaL, x0, x1, y0, y1)
        sH = _sigma_min(AH, BH, CH, xaH, yaH, x0, x1, y0, y1)
        iL = np.nonzero(ampL * np.exp(-sL) >= EPS)[0]
        iH = np.nonzero(ampH * np.exp(-sH) >= EPS)[0]
        return iL, iH

    def lanecount(sel):
        return len(sel[0]) + 4 * len(sel[1])

    # ---- shared program geometry ----
    segs_of_pos = []   # per position: list of (x0, x1, pxw, nchunk, ncar)
    units = []
    for p in range(NPOS):
        s16 = [select(k, POSW * p, POSW * (p + 1)) for k in range(NCORES)]
        if max(lanecount(s) for s in s16) <= 128:
            seglist = [(POSW * p, POSW * (p + 1), 512, s16)]
        else:
            seglist = [
                (POSW * p, POSW * p + 8, 256,
                 [select(k, POSW * p, POSW * p + 8) for k in range(NCORES)]),
                (POSW * p + 8, POSW * (p + 1), 256,
                 [select(k, POSW * p + 8, POSW * (p + 1))
                  for k in range(NCORES)])]
        segs = []
        ulist = []
        for sidx, (x0, x1, pxw, sels) in enumerate(seglist):
            nchunk = max(max(1, -(-lanecount(s) // 128)) for s in sels)
            ncar = max(-(-(4 * len(s[1])) // 128) for s in sels)
            segs.append((x0, x1, pxw, nchunk, ncar))
            for j in range(nchunk):
                ulist.append((pxw, j < ncar, sidx))
        segs_of_pos.append(segs)
        units.append(tuple(ulist))
    key = tuple(units)

    tot_units = sum(len(u) for u in units)
    CA = _BEND + 128 * tot_units \
        + 128 * sum(1 for u in units for (_, hc, _) in u if hc)
    pb_off_h = {}
    cbh = 0
    for p in range(NPOS):
        for j, (pxw, hc, sidx) in enumerate(units[p]):
            pb_off_h[(p, j)] = cbh
            cbh += 6 if (CFG.get("dimg", False) and hc) else 3
    CB = max(cbh, 512) + 67

    b16, q16, qcx16, qcy16 = _mk_basis(16)
    b8, q8, qcx8, qcy8 = _mk_basis(8)

    in_maps = []
    for k in range(NCORES):
        pa = np.zeros((12, CA), np.float64)
        pbv = np.zeros((128, CB), np.float64)
        pa[:, _B16:_B16 + 512] = b16
        pa[:, _BQ16:_BQ16 + 512] = q16
        pa[:, _B8:_B8 + 256] = b8
        pa[:, _BQ8:_BQ8 + 256] = q8
        Yc = ROWS * k + 16.0
        # column cursors mirror _build_program's offset assignment
        c_sig = _BEND
        c_ph = _BEND + 128 * tot_units
        ci = 0
        for p in range(NPOS):
            ju = 0
            for (x0, x1, pxw, nchunk, ncar) in segs_of_pos[p]:
                iL, iH = select(k, x0, x1)
                nLa, nHa = len(iL), len(iH)
                nl = 4 * nHa + nLa
                Xc = (x0 + x1) / 2.0
                qcx, qcy = (qcx16, qcy16) if pxw == 512 else (qcx8, qcy8)
                # vectorized per-lane quantities, carrier lanes first
                isc = np.zeros(nl, bool)
                isc[:4 * nHa] = True
                hrep = np.repeat(iH, G)
                A = np.concatenate([AH[hrep], AL[iL]])
                B = np.concatenate([BH[hrep], BL[iL]])
                C = np.concatenate([CH[hrep], CL[iL]])
                ax = Xc - np.concatenate([xaH[hrep], xaL[iL]])
                ay = Yc - np.concatenate([yaH[hrep], yaL[iL]])
                ws = np.stack([0.5 * A, B, 0.5 * C,
                               A * ax + B * ay, B * ax + C * ay,
                               0.5 * A * ax * ax + B * ax * ay
                               + 0.5 * C * ay * ay])           # [6, nl]
                hi = _bf16r(ws)
                sig12 = np.zeros((12, nl))
                sig12[0::2] = hi
                sig12[1::2] = ws - hi
                gl = np.arange(4 * nHa) % G
                fxc = fx[hrep, gl]
                fyc = fy[hrep, gl]
                base = fxc * ax[:4 * nHa] + fyc * ay[:4 * nHa]
                ph6 = np.zeros((6, 4 * nHa))
                ph6[0] = fxc
                ph6[1] = fyc
                for qq in range(4):
                    ph6[2 + qq] = base - np.round(base + fxc * qcx[qq]
                                                  + fyc * qcy[qq])
                ph6 *= 0.5   # half angle in cycle units for sin2pi
                if CFG.get("dimg", False):
                    colsE = np.concatenate([
                        colH[hrep] * wg[hrep, gl][:, None] * 255.0,
                        colL[iL] * 255.0])
                    colsM = np.concatenate([
                        -2.0 * colH[hrep] * wg[hrep, gl][:, None] * 255.0,
                        np.zeros((nLa, 3))])
                else:
                    cols = np.concatenate([
                        -2.0 * colH[hrep] * wg[hrep, gl][:, None] * 255.0,
                        colL[iL] * 255.0])                     # [nl, 3]
                for j in range(nchunk):
                    lo, hi_ = j * 128, min((j + 1) * 128, nl)
                    n = max(0, hi_ - lo)
                    so = c_sig
                    c_sig += 128
                    has_car = j < ncar
                    pa[10, so:so + 128] = 1000.0   # pad: sigma -> env 0
                    if n > 0:
                        pa[:, so:so + n] = sig12[:, lo:hi_]
                    if has_car:
                        po = c_ph
                        c_ph += 128
                        ncl = max(0, min(hi_, 4 * nHa) - lo)
                        if ncl > 0:
                            pa[0:6, po:po + ncl] = ph6[:, lo:lo + ncl]
                        # L lanes in this unit ride w=-env/2: color -2*col*255
                        if not CFG.get("dimg", False) and n > ncl:
                            cols[lo + ncl:hi_] *= -2.0
                    if n > 0:
                        bo = pb_off_h[(p, ju)]
                        if CFG.get("dimg", False) and has_car:
                            pbv[0:n, bo:bo + 3] = colsE[lo:hi_]
                            pbv[0:n, bo + 3:bo + 6] = colsM[lo:hi_]
                        elif CFG.get("dimg", False):
                            pbv[0:n, bo:bo + 3] = colsE[lo:hi_]
                        else:
                            pbv[0:n, bo:bo + 3] = cols[lo:hi_]
                    ci += 1
                    ju += 1
        in_maps.append({
            "pa": np.ascontiguousarray(pa, np.float32),
            "pb": np.ascontiguousarray(pbv.astype(np.float32)
                                       .astype(BF16NP)),
        })
    return in_maps, key


_GMAP = {}


def _gmap_from_key(key):
    """colmap[r, c] = output column holding geometric pixel (r, c)."""
    if key in _GMAP:
        return _GMAP[key]
    colmap = np.zeros((ROWS, W), np.int64)
    for p in range(NPOS):
        base = p * PPX
        off = 0
        for w in _seg_widths(key[p]):
            wcols = w // ROWS
            x0 = POSW * p + (off // ROWS)
            rr, ccc = np.meshgrid(np.arange(ROWS), np.arange(x0, x0 + wcols),
                                  indexing="ij")
            colmap[rr, ccc] = base + off + (rr * wcols + (ccc - x0))
            off += w
    _GMAP[key] = colmap.reshape(-1)
    return _GMAP[key]


def _assemble(results, key):
    gmap = _gmap_from_key(key)
    img = np.zeros((3, H, W), np.float32)
    for k in range(NCORES):
        o = np.asarray(results[k]["out"]).astype(np.float32) / 255.0
        img[:, k * ROWS:(k + 1) * ROWS, :] = o[:, gmap].reshape(3, ROWS, W)
    return img[None]


def kernel(**inputs):
    inputs = {k: np.asarray(v, np.float32) for k, v in inputs.items()}
    in_maps, key = _host_inputs(**inputs)
    if key not in _CACHE:
        _CACHE[key] = _build_program(key)
    nc = _CACHE[key]
    res = run_bass_kernel_spmd(nc, in_maps, list(range(NCORES)))
    return _assemble(res.results, key).astype(np.float32)


if __name__ == "__main__":
    import reference
    ins = {k: np.asarray(v) for k, v in reference.setup_inputs().items()}
    out = kernel(**ins)
    ref = np.asarray(reference.reference(**reference.setup_inputs()))
    rel = np.linalg.norm(out - ref) / np.linalg.norm(ref)
    print("Relative error:", rel)


# revision 38
# speedup vs baseline: 1.6692x; 1.0373x over previous
"""Trainium2 Bass kernel for mixed Gaussian/Gabor splat rasterization.

Problem: render 3072 plain 2D gaussians + 1024 gabor-modulated gaussians
(G=4 cosine carriers each) densely into a [1,3,256,256] image, clamp [0,1].

v3 design (carrier-major, host-heavy, adaptive blocks):
  - Pixels sharded across 8 cores: core k owns rows [32k, 32k+32), split into
    16 column POSITIONS of 16x32 px.  A position whose densest core has more
    than 128 overlapping lanes is split into two 8x32 SEGMENTS, so (for these
    inputs) every segment's lanes fit one 128-partition chunk and no pixel is
    rasterized twice.  Every engine op's cost scales with its free size
    (pixels), not partitions, so this halves the dense positions' work.
  - Lanes: a low gaussian is 1 lane; a high gaussian is 4 carrier lanes (one
    per gabor carrier), each carrying its own copy of the parent's sigma
    rows.  Selection is contribution-based: keep a gaussian in a segment if
    amp * exp(-sigma(closest point)) >= EPS, where amp is |color|max (low) or
    |color|max * sum|wg| (high).
  - All matmul lhsT data is precomputed on the host in f64:
      sigma [12,128]: bf16 hi/lo split weights [w0h,w0l..w5h,w5l] against
        basis rows [x2,x2,xy,xy,y2,y2,x,x,y,y,1,1] (basis values are exact in
        f32r), with the constant w5 folded in so Exp needs no bias.
      phase [6,128]: [fx,fy,o0..o3] half-angle cycles/px + per-quarter
        rounded offsets (one-hot quarter rows in the pixel basis) keep the
        activation argument within +-0.38 cycles.  The Sin activation is
        renamed to the HW table function Sin2pi (= sin(2*pi*x), measured
        exact for |x| <= 0.5) in the BIR JSON at serialization time;
        Sin2pi lives in the same act-table set as Exp ("exp_and_friends"),
        so the whole kernel needs ONE table load instead of two per pass.
  - cos via half angle with the -2 folded into the colors: pass A computes
    m = sin^2(pi t) (Sin -> bf16, square on DVE in 2x mode); pass B merges
    w = (m - 0.5) * env, and a carrier unit's single color matmul uses
    c = -2*col*wg*255 (carrier lanes) / -2*col*255 (low lanes riding a
    carrier unit, whose m is 0 so w = -env/2).  Pure-low units matmul their
    colors (col*255) directly against env.  Colors are pre-scaled by 255 so
    the clamp is one (max 0, min 255) tensor_scalar straight to uint8.
  - Positions are processed in PAIRS per activation (per-act fixed cost
    amortized); squares and merges are batched per group on DVE.  The image
    accumulates in PSUM tiles packing 3 positions at partition bases
    0/32/64 (zero-weight matmuls open/close each tile's accumulation
    group), so one clamp tensor_scalar serves 3 positions.
  - 2 input DMAs; output leaves via 16 small per-position DMAs on the SP
    queue (HWDGE), keeping Pool free.
"""

import math
import numpy as np

try:
    import concourse.bass as bass
except ImportError:
    import sys
    sys.path.insert(0, "/opt/trn_rl_repo")
    import concourse.bass as bass

import ml_dtypes
import concourse.tile as tile
from concourse import bacc, mybir
from concourse.bass_utils import run_bass_kernel_spmd

F32 = mybir.dt.float32
F32R = mybir.dt.float32r
BF16 = mybir.dt.bfloat16
U8 = mybir.dt.uint8
OP = mybir.AluOpType
AF = mybir.ActivationFunctionType

H = 256
W = 256
G = 4
NCORES = 8
ROWS = H // NCORES          # 32 rows per core
POSW = 16                   # columns per position
NPOS = W // POSW            # 16 positions per core
PPX = POSW * ROWS           # 512 px per position
PX = ROWS * W               # 8192 px per core
INV2PI = 1.0 / (2.0 * math.pi)
EPS = 5e-4                  # contribution cutoff amp*exp(-sigma_min)
BF16NP = ml_dtypes.bfloat16

_CACHE = {}

# Route Sin through the exp_and_friends activation table set: the HW table
# function sin2pi(x) = sin(2*pi*x) is exact for |x| <= 0.5 (measured) and
# lives in the same set as exp, so one table load serves the whole kernel.
# bass's enum has no Sin2pi, so we build with AF.Sin (scale chosen so the
# interpreter computes the same values) and rename the func in the BIR JSON
# at serialization time; insert_act_table_loads is steered by advertising
# Sin as a member of exp_and_friends (and of nothing else).
_TABLES_PATCHED = False


def _patch_act_tables():
    global _TABLES_PATCHED
    if _TABLES_PATCHED:
        return
    orig = bacc.get_activation_tables

    def patched(arch):
        t = orig(arch)
        out = {n: set(fs) for n, fs in t.items()}
        for n in out:
            out[n].discard(AF.Sin)
        out["exp_and_friends"].add(AF.Sin)
        return out

    bacc.get_activation_tables = patched
    _TABLES_PATCHED = True


# engine allocation knobs (sim-swept): merge/square engine per unit index:
# 'V' = DVE, 'P' = Pool, 'A' = alternate by unit; clamp_pair: one clamp per
# activation pair (else per position); quad_a: 4-position Sin batches.
CFG = {"merge": "V", "square": "V", "clamp_pair": False, "quad_a": False,
       "bsingle": False, "sigbufs": 2, "imgbufs": 2, "sqbatch": True,
       "noclose": True, "phbufs": 2, "sgbufs": 4, "envbufs": 5, "wbufs": 5}

# basis column offsets in pa: [basis16 | basisq16 | basis8 | basisq8 | lhsT..]
_B16, _BQ16, _B8, _BQ8, _BEND = 0, 512, 1024, 1280, 1536

# key: tuple over positions of unit tuples (pxw, has_car, seg_idx).
# Units are in segment order (left-to-right), chunks of a segment adjacent.
# The output pixel layout of a position is its segments' pixels in order,
# row-major within each segment; multi-chunk segments appear once.


def _seg_widths(posunits):
    seen = {}
    for (pxw, hc, sidx) in posunits:
        seen[sidx] = pxw
    return [seen[s] for s in sorted(seen)]


def _build_program(key, repeat=1, hwpatch=True):
    _patch_act_tables()
    nc = bacc.Bacc("TRN2", target_bir_lowering=False, debug=False)
    units = key
    tot_units = sum(len(u) for u in units)
    tot_car = sum(1 for u in units for (_, hc, _) in u if hc)
    CA = _BEND + 128 * tot_units + 128 * tot_car

    # column offsets (must match _host_inputs layout)
    sig_off = {}
    ph_off = {}
    slot_off = {}
    pb_off = {}
    nrow_t = {t: 32 * (min(3, NPOS - 3 * t) - 1) + 3
              for t in range((NPOS + 2) // 3)}
    tri_units = {t: 0 for t in nrow_t}
    c = _BEND
    cb = 0
    dimg = CFG.get("dimg", False)
    for p in range(NPOS):
        for j, (pxw, hc, sidx) in enumerate(units[p]):
            sig_off[(p, j)] = c
            c += 128
            pb_off[(p, j)] = cb
            cb += 6 if (dimg and hc) else 3
            tri_units[p // 3] += 1
    # pb: color cols per unit, padded to >=512 (zero-matmul rhs), plus a
    # 67-col zero block for the per-triple psum initializer lhsT
    CB = max(cb, 512) + 67
    pa = nc.declare_dram_parameter("pa", [12, CA], F32, isOutput=False)
    ZOFF = CB - 67
    pb = nc.declare_dram_parameter("pb", [128, CB], BF16, isOutput=False)
    out_ext = nc.declare_dram_parameter("out", [3, PX], U8, isOutput=True)
    scol = 0
    for p in range(NPOS):
        for j, (pxw, hc, sidx) in enumerate(units[p]):
            if hc:
                ph_off[(p, j)] = c
                c += 128
                slot_off[(p, j)] = scol
                scol += pxw
    assert c == CA
    mcols = max(1, scol)
    # env-psum width per position (units side by side, chunks duplicated)
    envw = {p: sum(pxw for (pxw, _, _) in units[p]) for p in range(NPOS)}
    carw = {p: sum(pxw for (pxw, hc, _) in units[p] if hc)
            for p in range(NPOS)}
    # geometric pixel offset of each segment inside the position's 512 px
    seg_px_off = {}
    for p in range(NPOS):
        o = 0
        for s, w in enumerate(_seg_widths(units[p])):
            seg_px_off[(p, s)] = o
            o += w
        assert o == PPX

    V = nc.vector
    S = nc.scalar
    T = nc.tensor
    PI = math.pi

    # position groups for act batching: greedy up to CFG bcap f32 psum cols
    # (and at most bmax positions), so light positions share an activation
    pairs = []
    p = 0
    bcap = CFG.get("bcap", 1024)
    bmax = CFG.get("bmax", 2)
    while p < NPOS:
        g = [p]
        w = envw[p]
        while p + len(g) < NPOS and len(g) < bmax \
                and w + envw[p + len(g)] <= bcap:
            w += envw[p + len(g)]
            g.append(p + len(g))
        pairs.append(tuple(g))
        p += len(g)
    # pass-A groups: up to 4 positions per Sin (psum cap 2048 f32 cols)
    agroups = []
    p = 0
    while p < NPOS:
        g = [p]
        w = carw[p]
        while p + len(g) < NPOS and w + carw[p + len(g)] <= 2048 \
                and len(g) < 4:
            w += carw[p + len(g)]
            g.append(p + len(g))
        agroups.append(tuple(g))
        p += len(g)

    with tile.TileContext(nc, pool_alloc_mode="queue") as tc:
        with tc.tile_pool(name="singles", bufs=1) as singles:
            pa_sb = singles.tile([12, CA], F32R)
            nc.gpsimd.dma_start(out=pa_sb, in_=pa[:])
            pb_sb = singles.tile([128, CB], BF16)
            nc.sync.dma_start(out=pb_sb, in_=pb[:])
            outu67 = singles.tile([67, (NPOS + 2) // 3 * PPX], U8)
            mbuf = singles.tile([128, mcols], BF16)

            def bq(pxw):
                return (pa_sb[0:6, _BQ16:_BQ16 + 512] if pxw == 512
                        else pa_sb[0:6, _BQ8:_BQ8 + 256])

            def bs(pxw):
                return (pa_sb[:, _B16:_B16 + 512] if pxw == 512
                        else pa_sb[:, _B8:_B8 + 256])

            for _rep in range(repeat):
                op = CFG.get("onepass", False)
                import contextlib
                stack = contextlib.ExitStack()
                php = stack.enter_context(tc.tile_pool(
                    name="ph", bufs=(1 if op else CFG.get("phbufs", 2)),
                    space="PSUM"))
                sgp = stack.enter_context(tc.tile_pool(
                    name="sgp", bufs=CFG.get("sgbufs", 2)))

                def do_a(pr):
                    cw = sum(carw[p] for p in pr)
                    if cw == 0:
                        return
                    ph = php.tile([128, cw], F32, name="ph", tag="ph")
                    sg = sgp.tile([128, cw], BF16, name="sg", tag="sg")
                    o = 0
                    offs = []
                    for p in pr:
                        for j, (pxw, hc, sidx) in enumerate(units[p]):
                            if not hc:
                                continue
                            po = ph_off[(p, j)]
                            T.matmul(ph[:, o:o + pxw],
                                     pa_sb[0:6, po:po + 128], bq(pxw),
                                     start=True, stop=True)
                            offs.append((p, j, o, pxw))
                            o += pxw
                    S.activation(sg, ph, AF.Sin,
                                 scale=(1.0 if hwpatch else 2.0 * PI))
                    sqE = {"V": V, "P": nc.gpsimd}.get(
                        CFG["square"], V if (pr[0] // 2) % 2 else nc.gpsimd)
                    if CFG.get("sqbatch") and offs:
                        so0 = slot_off[(offs[0][0], offs[0][1])]
                        sqE.tensor_tensor(out=mbuf[:, so0:so0 + cw],
                                          in0=sg, in1=sg, op=OP.mult)
                    else:
                        for ui, (p, j, o, pxw) in enumerate(offs):
                            so = slot_off[(p, j)]
                            sqE.tensor_tensor(out=mbuf[:, so:so + pxw],
                                              in0=sg[:, o:o + pxw],
                                              in1=sg[:, o:o + pxw],
                                              op=OP.mult)

                if not op:
                    for pr in (agroups if CFG["quad_a"] else pairs):
                        do_a(pr)
                    stack.close()
                    stack = contextlib.ExitStack()

                sigp = stack.enter_context(tc.tile_pool(
                    name="sig", bufs=CFG["sigbufs"], space="PSUM"))
                envp = stack.enter_context(tc.tile_pool(
                    name="envp", bufs=CFG.get("envbufs", 3)))
                wp = stack.enter_context(tc.tile_pool(
                    name="wp", bufs=CFG.get("wbufs", 4)))
                imgp = stack.enter_context(tc.tile_pool(
                    name="imgp", bufs=CFG["imgbufs"], space="PSUM"))

                def do_sig(pr):
                    tw = sum(envw[p] for p in pr)
                    sig = sigp.tile([128, tw], F32, name="sig", tag="sig")
                    o = 0
                    offs = []
                    for p in pr:
                        for j, (pxw, hc, sidx) in enumerate(units[p]):
                            so = sig_off[(p, j)]
                            T.matmul(sig[:, o:o + pxw],
                                     pa_sb[:, so:so + 128], bs(pxw),
                                     start=True, stop=True)
                            offs.append((p, j, o, pxw, hc, sidx))
                            o += pxw
                    return sig, offs, tw

                def do_rest(pr, sig, offs, tw):
                    env = envp.tile([128, tw], BF16, name="env", tag="env")
                    S.activation(env, sig, AF.Exp, scale=-1.0)
                    for p in pr:
                        t = p // 3
                        if t not in trip:
                            tl = imgp.tile([nrow_t[t], PPX], F32,
                                           name="img", tag="img")
                            T.matmul(tl[0:nrow_t[t], :],
                                     pb_sb[:, ZOFF:ZOFF + nrow_t[t]],
                                     pb_sb[:, 0:PPX],
                                     start=True, stop=False)
                            trip[t] = [tl, 0]
                    allcar = all(hc for (_, _, _, _, hc, _) in offs)
                    mgE = {"V": V, "P": nc.gpsimd}.get(
                        CFG["merge"], V if (pr[0] // 2) % 2 else nc.gpsimd)
                    wslices = {}
                    if allcar:
                        wt = wp.tile([128, tw], BF16, name="w", tag="w")
                        so0 = slot_off[(offs[0][0], offs[0][1])]
                        mgE.scalar_tensor_tensor(
                            out=wt, in0=mbuf[:, so0:so0 + tw],
                            scalar=-0.5, in1=env, op0=OP.add, op1=OP.mult)
                        for (p, j, o, pxw, hc, sidx) in offs:
                            wslices[(p, j)] = wt[:, o:o + pxw]
                    else:
                        for (p, j, o, pxw, hc, sidx) in offs:
                            if hc:
                                so = slot_off[(p, j)]
                                w = wp.tile([128, pxw], BF16, name="w",
                                            tag="w")
                                mgE.scalar_tensor_tensor(
                                    out=w, in0=mbuf[:, so:so + pxw],
                                    scalar=-0.5, in1=env[:, o:o + pxw],
                                    op0=OP.add, op1=OP.mult)
                                wslices[(p, j)] = w
                            else:
                                wslices[(p, j)] = env[:, o:o + pxw]
                    for (p, j, o, pxw, hc, sidx) in offs:
                        t, s = divmod(p, 3)
                        tile3, nm = trip[t]
                        trip[t][1] = nm + 1
                        xo = seg_px_off[(p, sidx)]
                        bo = pb_off[(p, j)]
                        lastmm = (CFG.get("noclose")
                                  and trip[t][1] == tri_units[t])
                        T.matmul(tile3[32 * s:32 * s + 3, xo:xo + pxw],
                                 pb_sb[:, bo:bo + 3], wslices[(p, j)],
                                 start=False, stop=lastmm)
                    for p in pr:
                        t = p // 3
                        if t in trip and trip[t][1] == tri_units[t]:
                            tile3, _ = trip.pop(t)
                            nrow = nrow_t[t]
                            if not CFG.get("noclose"):
                                T.matmul(tile3[0:nrow, :],
                                         pb_sb[:, ZOFF:ZOFF + nrow],
                                         pb_sb[:, 0:PPX],
                                         start=False, stop=True)
                            V.tensor_scalar(
                                outu67[0:nrow, t * PPX:(t + 1) * PPX],
                                tile3, 0.0, 255.0, OP.max, OP.min)
                            for ss in range(min(3, NPOS - 3 * t)):
                                nc.sync.dma_start(
                                    out=out_ext[:, (3 * t + ss) * PPX:
                                                (3 * t + ss + 1) * PPX],
                                    in_=outu67[32 * ss:32 * ss + 3,
                                               t * PPX:(t + 1) * PPX])

                trip = {}
                prev = None
                for pr in pairs:
                    if op:
                        do_a(pr)
                    cur = (pr,) + do_sig(pr)
                    if prev is not None:
                        do_rest(*prev)
                    prev = cur
                do_rest(*prev)
                stack.close()
    nc.finalize()
    if hwpatch:
        patched = nc.to_json_bytes().replace(b'"func":"Sin"',
                                             b'"func":"Sin2pi"')
        nc.to_json_bytes = lambda: patched
    return nc


def _planes(mu, chol, feat, opac):
    """Global per-gaussian conic + color, f64."""
    mu = np.asarray(mu, np.float64)
    chol = np.asarray(chol, np.float64)
    m = np.tanh(mu)
    xa = (m[:, 0] + 1.0) * 0.5 * W
    ya = (m[:, 1] + 1.0) * 0.5 * H
    l1 = chol[:, 0] + 0.5
    l2 = chol[:, 1]
    l3 = chol[:, 2] + 0.5
    sxx = l1 * l1
    sxy = l1 * l2
    syy = l2 * l2 + l3 * l3
    det = sxx * syy - sxy * sxy
    A = syy / det
    B = -sxy / det
    C = sxx / det
    colf = np.asarray(feat, np.float64) * np.asarray(opac, np.float64)
    return xa, ya, A, B, C, colf


def _bf16r(w):
    return np.asarray(w, np.float32).astype(BF16NP).astype(np.float64)


def _sigma_min(A, B, C, xa, ya, x0, x1, y0, y1):
    cx = np.clip(xa, x0, x1)
    cy = np.clip(ya, y0, y1)
    dx = cx - xa
    dy = cy - ya
    return 0.5 * A * dx * dx + B * dx * dy + 0.5 * C * dy * dy


def _mk_basis(w):
    rr, ccc = np.meshgrid(np.arange(ROWS), np.arange(w), indexing="ij")
    xc = (ccc - (w - 1) / 2.0).ravel()
    yc = (rr - 15.5).ravel()
    b = np.zeros((12, w * ROWS))
    b[0] = b[1] = xc * xc
    b[2] = b[3] = xc * yc
    b[4] = b[5] = yc * yc
    b[6] = b[7] = xc
    b[8] = b[9] = yc
    b[10] = b[11] = 1.0
    q = np.zeros((12, w * ROWS))
    q[0] = xc
    q[1] = yc
    xh = (xc >= 0).astype(np.float64)
    yh = (yc >= 0).astype(np.float64)
    for qq in range(4):
        qx, qy = divmod(qq, 2)
        q[2 + qq] = ((xh == qx) & (yh == qy)).astype(np.float64)
    qcx = np.array([-w / 4.0, -w / 4.0, w / 4.0, w / 4.0])
    qcy = np.array([-8.0, 8.0, -8.0, 8.0])
    return b, q, qcx, qcy


def _host_inputs(low_mu, high_mu, low_chol, high_chol, low_feat, high_feat,
                 low_opac, high_opac, gabor_freqs, gabor_weights):
    xaL, yaL, AL, BL, CL, colL = _planes(low_mu, low_chol, low_feat, low_opac)
    xaH, yaH, AH, BH, CH, colH = _planes(high_mu, high_chol, high_feat,
                                         high_opac)
    gf = np.asarray(gabor_freqs, np.float64)
    gw = np.asarray(gabor_weights, np.float64)
    nH = xaH.shape[0]
    fx = gf[:, 0].reshape(nH, G) * INV2PI   # cycles / px
    fy = gf[:, 1].reshape(nH, G) * INV2PI
    wg = gw[:, 0].reshape(nH, G)
    ampL = np.abs(colL).max(1)
    ampH = np.abs(colH).max(1) * np.abs(wg).sum(1)

    def select(k, x0, x1):
        y0, y1 = ROWS * k, ROWS * (k + 1)
        sL = _sigma_min(AL, BL, CL, xaL, yaL, x0, x1, y0, y1)
        sH = _sigma_min(AH, BH, CH, xaH, yaH, x0, x1, y0, y1)
        iL = np.nonzero(ampL * np.exp(-sL) >= EPS)[0]
        iH = np.nonzero(ampH * np.exp(-sH) >= EPS)[0]
        return iL, iH

    def lanecount(sel):
        return len(sel[0]) + 4 * len(sel[1])

    # ---- shared program geometry ----
    segs_of_pos = []   # per position: list of (x0, x1, pxw, nchunk, ncar)
    units = []
    for p in range(NPOS):
        s16 = [select(k, POSW * p, POSW * (p + 1)) for k in range(NCORES)]
        if max(lanecount(s) for s in s16) <= 128:
            seglist = [(POSW * p, POSW * (p + 1), 512, s16)]
        else:
            seglist = [
                (POSW * p, POSW * p + 8, 256,
                 [select(k, POSW * p, POSW * p + 8) for k in range(NCORES)]),
                (POSW * p + 8, POSW * (p + 1), 256,
                 [select(k, POSW * p + 8, POSW * (p + 1))
                  for k in range(NCORES)])]
        segs = []
        ulist = []
        for sidx, (x0, x1, pxw, sels) in enumerate(seglist):
            nchunk = max(max(1, -(-lanecount(s) // 128)) for s in sels)
            ncar = max(-(-(4 * len(s[1])) // 128) for s in sels)
            segs.append((x0, x1, pxw, nchunk, ncar))
            for j in range(nchunk):
                ulist.append((pxw, j < ncar, sidx))
        segs_of_pos.append(segs)
        units.append(tuple(ulist))
    key = tuple(units)

    tot_units = sum(len(u) for u in units)
    CA = _BEND + 128 * tot_units \
        + 128 * sum(1 for u in units for (_, hc, _) in u if hc)
    pb_off_h = {}
    cbh = 0
    for p in range(NPOS):
        for j, (pxw, hc, sidx) in enumerate(units[p]):
            pb_off_h[(p, j)] = cbh
            cbh += 6 if (CFG.get("dimg", False) and hc) else 3
    CB = max(cbh, 512) + 67

    b16, q16, qcx16, qcy16 = _mk_basis(16)
    b8, q8, qcx8, qcy8 = _mk_basis(8)

    in_maps = []
    for k in range(NCORES):
        pa = np.zeros((12, CA), np.float64)
        pbv = np.zeros((128, CB), np.float64)
        pa[:, _B16:_B16 + 512] = b16
        pa[:, _BQ16:_BQ16 + 512] = q16
        pa[:, _B8:_B8 + 256] = b8
        pa[:, _BQ8:_BQ8 + 256] = q8
        Yc = ROWS * k + 16.0
        # column cursors mirror _build_program's offset assignment
        c_sig = _BEND
        c_ph = _BEND + 128 * tot_units
        ci = 0
        for p in range(NPOS):
            ju = 0
            for (x0, x1, pxw, nchunk, ncar) in segs_of_pos[p]:
                iL, iH = select(k, x0, x1)
                nLa, nHa = len(iL), len(iH)
                nl = 4 * nHa + nLa
                Xc = (x0 + x1) / 2.0
                qcx, qcy = (qcx16, qcy16) if pxw == 512 else (qcx8, qcy8)
                # vectorized per-lane quantities, carrier lanes first
                isc = np.zeros(nl, bool)
                isc[:4 * nHa] = True
                hrep = np.repeat(iH, G)
                A = np.concatenate([AH[hrep], AL[iL]])
                B = np.concatenate([BH[hrep], BL[iL]])
                C = np.concatenate([CH[hrep], CL[iL]])
                ax = Xc - np.concatenate([xaH[hrep], xaL[iL]])
                ay = Yc - np.concatenate([yaH[hrep], yaL[iL]])
                ws = np.stack([0.5 * A, B, 0.5 * C,
                               A * ax + B * ay, B * ax + C * ay,
                               0.5 * A * ax * ax + B * ax * ay
                               + 0.5 * C * ay * ay])           # [6, nl]
                hi = _bf16r(ws)
                sig12 = np.zeros((12, nl))
                sig12[0::2] = hi
                sig12[1::2] = ws - hi
                gl = np.arange(4 * nHa) % G
                fxc = fx[hrep, gl]
                fyc = fy[hrep, gl]
                base = fxc * ax[:4 * nHa] + fyc * ay[:4 * nHa]
                ph6 = np.zeros((6, 4 * nHa))
                ph6[0] = fxc
                ph6[1] = fyc
                for qq in range(4):
                    ph6[2 + qq] = base - np.round(base + fxc * qcx[qq]
                                                  + fyc * qcy[qq])
                ph6 *= 0.5   # half angle in cycle units for sin2pi
                if CFG.get("dimg", False):
                    colsE = np.concatenate([
                        colH[hrep] * wg[hrep, gl][:, None] * 255.0,
                        colL[iL] * 255.0])
                    colsM = np.concatenate([
                        -2.0 * colH[hrep] * wg[hrep, gl][:, None] * 255.0,
                        np.zeros((nLa, 3))])
                else:
                    cols = np.concatenate([
                        -2.0 * colH[hrep] * wg[hrep, gl][:, None] * 255.0,
                        colL[iL] * 255.0])                     # [nl, 3]
                for j in range(nchunk):
                    lo, hi_ = j * 128, min((j + 1) * 128, nl)
                    n = max(0, hi_ - lo)
                    so = c_sig
                    c_sig += 128
                    has_car = j < ncar
                    pa[10, so:so + 128] = 1000.0   # pad: sigma -> env 0
                    if n > 0:
                        pa[:, so:so + n] = sig12[:, lo:hi_]
                    if has_car:
                        po = c_ph
                        c_ph += 128
                        ncl = max(0, min(hi_, 4 * nHa) - lo)
                        if ncl > 0:
                            pa[0:6, po:po + ncl] = ph6[:, lo:lo + ncl]
                        # L lanes in this unit ride w=-env/2: color -2*col*255
                        if not CFG.get("dimg", False) and n > ncl:
                            cols[lo + ncl:hi_] *= -2.0
                    if n > 0:
                        bo = pb_off_h[(p, ju)]
                        if CFG.get("dimg", False) and has_car:
                            pbv[0:n, bo:bo + 3] = colsE[lo:hi_]
                            pbv[0:n, bo + 3:bo + 6] = colsM[lo:hi_]
                        elif CFG.get("dimg", False):
                            pbv[0:n, bo:bo + 3] = colsE[lo:hi_]
                        else:
                            pbv[0:n, bo:bo + 3] = cols[lo:hi_]
                    ci += 1
                    ju += 1
        in_maps.append({
            "pa": np.ascontiguousarray(pa, np.float32),
            "pb": np.ascontiguousarray(pbv.astype(np.float32)
                                       .astype(BF16NP)),
        })
    return in_maps, key


_GMAP = {}


def _gmap_from_key(key):
    """colmap[r, c] = output column holding geometric pixel (r, c)."""
    if key in _GMAP:
        return _GMAP[key]
    colmap = np.zeros((ROWS, W), np.int64)
    for p in range(NPOS):
        base = p * PPX
        off = 0
        for w in _seg_widths(key[p]):
            wcols = w // ROWS
            x0 = POSW * p + (off // ROWS)
            rr, ccc = np.meshgrid(np.arange(ROWS), np.arange(x0, x0 + wcols),
                                  indexing="ij")
            colmap[rr, ccc] = base + off + (rr * wcols + (ccc - x0))
            off += w
    _GMAP[key] = colmap.reshape(-1)
    return _GMAP[key]


def _assemble(results, key):
    gmap = _gmap_from_key(key)
    img = np.zeros((3, H, W), np.float32)
    for k in range(NCORES):
        o = np.asarray(results[k]["out"]).astype(np.float32) / 255.0
        img[:, k * ROWS:(k + 1) * ROWS, :] = o[:, gmap].reshape(3, ROWS, W)
    return img[None]


def kernel(**inputs):
    inputs = {k: np.asarray(v, np.float32) for k, v in inputs.items()}
    in_maps, key = _host_inputs(**inputs)
    if key not in _CACHE:
        _CACHE[key] = _build_program(key)
    nc = _CACHE[key]
    res = run_bass_kernel_spmd(nc, in_maps, list(range(NCORES)))
    return _assemble(res.results, key).astype(np.float32)


if __name__ == "__main__":
    import reference
    ins = {k: np.asarray(v) for k, v in reference.setup_inputs().items()}
    out = kernel(**ins)
    ref = np.asarray(reference.reference(**reference.setup_inputs()))
    rel = np.linalg.norm(out - ref) / np.linalg.norm(ref)
    print("Relative error:", rel)
